# revision 5
# baseline (speedup 1.0000x reference)
"""Trainium2 Bass kernel (v6: finer int8 scales, int8 sbf, in-graph zeros) for DimeNet-style Interaction block (gnn_message_passing).

v3 = v2 (AllGather + device-side dma_gather, fp16 transfers) plus:
  - iota / f32-identity embedded in the NEFF via inline_tensor (no H2D)
  - module-level cache of the compiled program AND the jitted PJRT callable:
    repeat executions skip jit re-tracing (~1.7s) and create the donated
    zero output buffers on-device instead of shipping 27MB of zeros H2D.

Transfer budget per exec: ~42MB H2D + ~27MB D2H over the ~45MB/s axon
tunnel -> ~1.5s steady-state (baseline shipped ~385MB -> 7.5s).
"""
import os
import numpy as np

H, B, NR, NS = 128, 8, 6, 7
P = 128
N_CORES = 8
E_TOT = 100000
EPC = E_TOT // N_CORES          # 12500 edges per core
ROWS_PC = 16384                 # padded x_kj rows per core (AllGather stripe)
CHUNK = 32768                   # dma_gather int16 index range per chunk
EP_N = 512

_CACHE = {}


def host_prep(x, radial, sbf_all, e_from, e_to):
    """Per-core packed tensors + meta. All numpy, vectorized."""
    sbf_scale = np.maximum(np.abs(sbf_all).max(0) / 127.0, 1e-20)  # [B] f64
    perm = np.argsort(e_to, kind="stable")
    tos = e_to[perm].astype(np.int64)
    frs = e_from[perm].astype(np.int64)

    cores = []
    NB_max = 0
    for c in range(N_CORES):
        lo, hi = c * EPC, (c + 1) * EPC
        t0, t1 = np.searchsorted(tos, lo), np.searchsorted(tos, hi)
        lt = tos[t0:t1] - lo
        fr = frs[t0:t1]
        # gather row = core(from)*16384 + slotpos, slotpos < 16384
        # -> chunk(row) = row >> 15 = core(from) // 2 exactly.
        chk = (fr // EPC) // 2
        cnt = np.bincount(lt * 4 + chk, minlength=EPC * 4).reshape(EPC, 4)
        Ccum = np.concatenate([np.zeros((1, 4), np.int64), np.cumsum(cnt, 0)], 0)
        s, starts = 0, []
        while s < EPC:
            ends = [np.searchsorted(Ccum[:, k], Ccum[s, k] + P, side="right") - 1
                    for k in range(4)]
            end = min(s + P, *ends)
            assert end > s, "single edge exceeds 128 triplets in one chunk"
            starts.append(s)
            s = end
        starts = np.asarray(starts + [EPC], np.int64)
        NB_max = max(NB_max, len(starts) - 1)
        cores.append(dict(lo=lo, hi=hi, lt=lt, fr=fr, chk=chk,
                          tsl=perm[t0:t1], starts=starts, nb=len(starts) - 1))

    NB = -(-NB_max // 8) * 8
    W_S = NB * P
    NSUB = 4 * NB

    slotpos = np.empty(E_TOT, np.int64)
    for core in cores:
        starts, nb = core["starts"], core["nb"]
        widths = np.diff(starts)
        blk_of_edge = np.repeat(np.arange(nb), widths)
        cov_lo = starts[:-1]
        sp = blk_of_edge * P + (np.arange(EPC) - cov_lo[blk_of_edge])
        slotpos[core["lo"]:core["hi"]] = sp
        core["blk_of_edge"] = blk_of_edge
        core["cov_lo"] = cov_lo
        core["slot_local"] = sp
    grow = (np.arange(E_TOT) // EPC) * ROWS_PC + slotpos

    for core in cores:
        lt, fr, tsl = core["lt"], core["fr"], core["tsl"]
        blk = core["blk_of_edge"][lt]
        gid = blk * 4 + core["chk"]
        order = np.argsort(gid, kind="stable")
        gid_s = gid[order]
        first_idx = np.concatenate([[0], np.flatnonzero(np.diff(gid_s)) + 1])
        counts = np.diff(np.concatenate([first_idx, [len(gid_s)]]))
        assert counts.max() <= P, counts.max()
        rank = np.arange(len(gid_s)) - np.repeat(first_idx, counts)
        dst = gid_s * P + rank
        tri = order
        idx_arr = np.zeros(NSUB * P, np.int16)
        tol_arr = np.full(NSUB * P, 255.0, np.float16)
        sbf_arr = np.zeros((NSUB * P, B), np.int8)
        idx_arr[dst] = (grow[fr[tri]] & (CHUNK - 1)).astype(np.int16)
        tol_arr[dst] = (lt[tri] - core["cov_lo"][blk[tri]]).astype(np.float16)
        sbf_arr[dst] = np.clip(np.round(sbf_all[tsl[tri]] / sbf_scale[None, :]),
                               -127, 127).astype(np.int8)
        core["idx16"] = np.ascontiguousarray(
            idx_arr.reshape(NSUB, 8, 16).transpose(2, 0, 1).reshape(16, NSUB * 8))
        core["tol"] = np.ascontiguousarray(tol_arr.reshape(NSUB, P).T)
        core["sbf"] = np.ascontiguousarray(
            sbf_arr.reshape(NSUB, P, B).transpose(1, 0, 2).reshape(P, NSUB * B))
        rs = np.zeros((W_S, NR), np.float16)
        sl = core["slot_local"]
        rs[sl] = radial[core["lo"]:core["hi"]].astype(np.float16)
        core["rad_slots"] = np.ascontiguousarray(rs.T)
        # int8 x with per-feature-per-block scales (dequantized on device)
        xc = x[core["lo"]:core["hi"]]                    # [EPC, H]
        xf = np.zeros((W_S, H), np.float32)
        xf[sl] = xc
        xf3 = xf.reshape(NB, P, H)
        sc = np.maximum(np.abs(xf3).max(1) / 127.0, 1e-20).astype(np.float32)  # [NB, H]
        xq3 = np.clip(np.round(xf3 / sc[:, None, :]), -127, 127).astype(np.int8)
        core["x_q"] = np.ascontiguousarray(xq3.reshape(W_S, H).T)   # [H, W_S]
        core["xsc"] = np.ascontiguousarray(sc.T)         # [H, NB] f32
    return cores, dict(NB=NB, W_S=W_S, NSUB=NSUB, sbf_scale=sbf_scale)


def build_program(NB, W_S, NSUB):
    import concourse.tile as tile
    from concourse import bacc, mybir

    f32 = mybir.dt.float32
    f16 = mybir.dt.float16
    i16 = mybir.dt.int16
    AF = mybir.ActivationFunctionType
    ALU = mybir.AluOpType

    nc = bacc.Bacc(None, target_bir_lowering=False)
    CW16 = P + B * H + 8 * H + P           # w_from|W2|epw|w_rbf
    i8 = mybir.dt.int8
    c16_d = nc.dram_tensor("c16sh", [16, CW16], f16, kind="ExternalInput")
    c32_d = nc.dram_tensor("c32sh", [16, P + 9], f32, kind="ExternalInput")  # b_bcast|biases
    sbf_d = nc.dram_tensor("sbf", [P, B * NSUB], mybir.dt.int8, kind="ExternalInput")
    tol_d = nc.dram_tensor("tol", [P, NSUB], f16, kind="ExternalInput")
    idx_d = nc.dram_tensor("idx", [16, 8 * NSUB], i16, kind="ExternalInput")
    xq_d = nc.dram_tensor("x_q", [P, W_S], i8, kind="ExternalInput")
    xsc_d = nc.dram_tensor("xsc", [P, NB], f32, kind="ExternalInput")
    rad_d = nc.dram_tensor("rad_slots", [NR, W_S], f16, kind="ExternalInput")
    out_d = nc.dram_tensor("out_q", [P, W_S], i8, kind="ExternalOutput")
    osc_d = nc.dram_tensor("out_sc", [P, NB], f32, kind="ExternalOutput")
    iota_c = nc.inline_tensor(
        np.tile(np.arange(P, dtype=np.float16), (P, 1)), name="iota_c")
    tiny_c = nc.inline_tensor(np.full((P, 1), 1e-12, np.float32), name="tiny_c")
    idf32_c = nc.inline_tensor(np.eye(P, dtype=np.float32), name="idf32_c")

    with tile.TileContext(nc) as tc:
        with (
            tc.tile_pool(name="consts", bufs=1) as cp,
            tc.tile_pool(name="persist", bufs=1) as pp,
            tc.tile_pool(name="dram", bufs=1, space="DRAM") as dr,
        ):
            # weights arrive sharded (1/8 rows per core): AllGather on device
            c16b = dr.tile([16, CW16], f16)
            nc.gpsimd.dma_start(out=c16b[:, :], in_=c16_d[:, :])
            c16f = dr.tile([P, CW16], f16)
            nc.gpsimd.collective_compute(
                "AllGather", mybir.AluOpType.bypass,
                replica_groups=[list(range(N_CORES))],
                ins=[c16b[:, :].opt()], outs=[c16f[:, :].opt()])
            c32b = dr.tile([16, P + 9], f32)
            nc.gpsimd.dma_start(out=c32b[:, :], in_=c32_d[:, :])
            c32f = dr.tile([P, P + 9], f32)
            nc.gpsimd.collective_compute(
                "AllGather", mybir.AluOpType.bypass,
                replica_groups=[list(range(N_CORES))],
                ins=[c32b[:, :].opt()], outs=[c32f[:, :].opt()])
            c16 = cp.tile([P, CW16], f16)
            nc.gpsimd.dma_start(out=c16[:], in_=c16f[:, :])
            c32 = cp.tile([P, P + 9], f32)
            nc.gpsimd.dma_start(out=c32[:], in_=c32f[:, :])
            iota_t = cp.tile([P, P], f16)
            nc.gpsimd.dma_start(out=iota_t[:], in_=iota_c[:, :])
            idf32_t = cp.tile([P, P], f32)
            nc.gpsimd.dma_start(out=idf32_t[:], in_=idf32_c[:, :])
            sbf_t = cp.tile([P, B * NSUB], mybir.dt.int8)
            nc.gpsimd.dma_start(out=sbf_t[:], in_=sbf_d[:, :])
            tol_t = cp.tile([P, NSUB], f16)
            nc.gpsimd.dma_start(out=tol_t[:], in_=tol_d[:, :])
            xq_t = cp.tile([P, W_S], i8)
            nc.gpsimd.dma_start(out=xq_t[:], in_=xq_d[:, :])
            xsc_t = cp.tile([P, NB], f32)
            nc.gpsimd.dma_start(out=xsc_t[:], in_=xsc_d[:, :])
            xs_t = cp.tile([P, W_S], f16)
            for q in range(NB):
                q0 = q * P
                nc.scalar.activation(out=xs_t[:, q0:q0 + P], in_=xq_t[:, q0:q0 + P],
                                     func=AF.Copy, scale=xsc_t[:, q:q + 1])
            rad_t = cp.tile([NR, W_S], f16)
            nc.gpsimd.dma_start(out=rad_t[:], in_=rad_d[:, :])
            tiny_t = cp.tile([P, 1], f32)
            nc.gpsimd.dma_start(out=tiny_t[:], in_=tiny_c[:, :])
            osc_big = pp.tile([P, NB], f32)
            idx_t = cp.tile([P, 8 * NSUB], i16)
            for k in range(8):
                nc.gpsimd.dma_start(out=idx_t[16 * k:16 * k + 16, :], in_=idx_d[:, :])

            w_from_t = c16[:, 0:P]
            W2_t = c16[:, P:P + B * H]
            epw_t = c16[:, P + B * H:P + B * H + 8 * H]
            w_rbf_t = c16[0:NR, P + B * H + 8 * H:CW16]
            b_bcast = c32[:, 0:P]
            bias_t = c32[:, P:P + 9]

            aggT_big = pp.tile([P, W_S], f16)
            xkj_mine = dr.tile([ROWS_PC, H], f16)
            xkj_all = dr.tile([ROWS_PC * N_CORES, H], f16)

            # ---------------- stage 1: x_kj rows ----------------
            with (
                tc.tile_pool(name="s1w", bufs=3) as s1w,
                tc.tile_pool(name="s1p", bufs=2, space="PSUM") as s1p,
            ):
                for b in range(NB):
                    c0 = b * P
                    xw_p = s1p.tile([P, P], f32, tag="xw")
                    nc.tensor.matmul(out=xw_p[:], lhsT=xs_t[:, c0:c0 + P],
                                     rhs=w_from_t, start=True, stop=True)
                    rb_p = s1p.tile([P, P], f32, tag="rb")
                    nc.tensor.matmul(out=rb_p[:], lhsT=rad_t[:, c0:c0 + P],
                                     rhs=w_rbf_t, start=True, stop=True)
                    t1 = s1w.tile([P, P], f32, tag="t1")
                    nc.vector.tensor_tensor(out=t1[:], in0=xw_p[:], in1=b_bcast,
                                            op=ALU.add)
                    t2 = s1w.tile([P, P], f32, tag="t2")
                    nc.scalar.activation(out=t2[:], in_=t1[:], func=AF.Silu)
                    xkj = s1w.tile([P, P], f16, tag="xkj")
                    nc.vector.tensor_tensor(out=xkj[:], in0=t2[:], in1=rb_p[:],
                                            op=ALU.mult)
                    nc.gpsimd.dma_start(out=xkj_mine[c0:c0 + P, :], in_=xkj[:])

            # ---------------- AllGather ----------------
            nc.gpsimd.collective_compute(
                "AllGather", mybir.AluOpType.bypass,
                replica_groups=[list(range(N_CORES))],
                ins=[xkj_mine[:, :].opt()], outs=[xkj_all[:, :].opt()],
            )

            # ---------------- main loop ----------------
            with (
                tc.tile_pool(name="mg", bufs=4) as mg,
                tc.tile_pool(name="mw", bufs=3) as mw,
                tc.tile_pool(name="ptmp", bufs=1, space="PSUM") as ptmp,
                tc.tile_pool(name="pagg", bufs=2, space="PSUM") as pagg,
                tc.tile_pool(name="ptr", bufs=1, space="PSUM") as ptr,
            ):
                for b in range(NB):
                    agg_p = pagg.tile([P, P], f32, tag="agg")
                    for s in range(4):
                        sub = b * 4 + s
                        g3 = mg.tile([P, 1, P], f16, tag="g")
                        nc.gpsimd.dma_gather(
                            out_ap=g3[:],
                            in_ap=xkj_all[s * CHUNK:(s + 1) * CHUNK, :],
                            idxs_ap=idx_t[:, sub * 8:sub * 8 + 8],
                            num_idxs=P, num_idxs_reg=P,
                            elem_size=H, transpose=True)
                        g = g3[:].squeeze(1)
                        tmpA = ptmp.tile([P, 4 * H], f32, tag="tmpA")
                        nc.tensor.matmul(out=tmpA[:], lhsT=g,
                                         rhs=W2_t[:, 0:4 * H], start=True, stop=True)
                        tmpB = ptmp.tile([P, 4 * H], f32, tag="tmpB")
                        nc.tensor.matmul(out=tmpB[:], lhsT=g,
                                         rhs=W2_t[:, 4 * H:8 * H], start=True, stop=True)
                        sc = mw.tile([P, B], f32, tag="sc")
                        nc.scalar.activation(out=sc[:],
                                             in_=sbf_t[:, sub * B:(sub + 1) * B],
                                             func=AF.Copy)
                        S = mw.tile([P, P], f16, tag="S")
                        nc.vector.tensor_tensor(
                            out=S[:],
                            in0=tol_t[:, sub:sub + 1].to_broadcast([P, P]),
                            in1=iota_t[:], op=ALU.is_equal)
                        tmpS = mw.tile([P, B * H], f16, tag="tmpS")
                        for j in range(B):
                            src = tmpA[:, j * H:(j + 1) * H] if j < 4 else \
                                  tmpB[:, (j - 4) * H:(j - 3) * H]
                            dstp = tmpS[:, j * H:(j + 1) * H]
                            scj = sc[:, j:j + 1]
                            if j % 2 == 0:
                                nc.scalar.activation(out=dstp, in_=src, func=AF.Copy,
                                                     scale=scj)
                            else:
                                nc.vector.tensor_tensor(
                                    out=dstp, in0=src,
                                    in1=scj.to_broadcast([P, H]), op=ALU.mult)
                        for j in range(B):
                            nc.tensor.matmul(out=agg_p[:], lhsT=S[:],
                                             rhs=tmpS[:, j * H:(j + 1) * H],
                                             start=(s == 0 and j == 0),
                                             stop=(s == 3 and j == B - 1),
                                             skip_group_check=True)
                    agg_s = mw.tile([P, P], f32, tag="agg_s")
                    nc.scalar.activation(out=agg_s[:], in_=agg_p[:], func=AF.Copy)
                    aggT_p = ptr.tile([P, P], f32, tag="aggT")
                    nc.tensor.transpose(out=aggT_p[:], in_=agg_s[:], identity=idf32_t[:])
                    nc.vector.tensor_copy(out=aggT_big[:, b * P:(b + 1) * P],
                                          in_=aggT_p[:])

            # ---------------- epilogue ----------------
            with (
                tc.tile_pool(name="ew", bufs=2) as ew,
                tc.tile_pool(name="ep", bufs=4, space="PSUM") as ep,
            ):
                def ep_mm(lhs_idx, rhs_ap):
                    pt = ep.tile([P, EP_N], f32, tag="ep_p")
                    nc.tensor.matmul(out=pt[:],
                                     lhsT=epw_t[:, lhs_idx * H:(lhs_idx + 1) * H],
                                     rhs=rhs_ap, start=True, stop=True)
                    return pt

                def ep_silu(pt, bias_idx, tag):
                    t = ew.tile([P, EP_N], f16, tag=tag)
                    nc.scalar.activation(out=t[:], in_=pt[:], func=AF.Silu,
                                         bias=bias_t[:, bias_idx:bias_idx + 1],
                                         scale=1.0)
                    return t

                for eb in range(W_S // EP_N):
                    c0 = eb * EP_N
                    x_sl = xs_t[:, c0:c0 + EP_N]
                    xji = ep_silu(ep_mm(0, x_sl), 1, "xji")
                    h = ew.tile([P, EP_N], f16, tag="h")
                    nc.vector.tensor_tensor(out=h[:], in0=xji[:],
                                            in1=aggT_big[:, c0:c0 + EP_N], op=ALU.add)
                    t1 = ep_silu(ep_mm(1, h[:]), 2, "t1")
                    t2 = ep_silu(ep_mm(2, t1[:]), 3, "t2")
                    h2 = ew.tile([P, EP_N], f16, tag="h2")
                    nc.vector.tensor_tensor(out=h2[:], in0=h[:], in1=t2[:], op=ALU.add)
                    l1 = ep_silu(ep_mm(3, h2[:]), 4, "l1")
                    h3 = ew.tile([P, EP_N], f16, tag="h3")
                    nc.vector.tensor_tensor(out=h3[:], in0=l1[:], in1=x_sl, op=ALU.add)
                    t3 = ep_silu(ep_mm(4, h3[:]), 5, "t3")
                    t4 = ep_silu(ep_mm(5, t3[:]), 6, "t4")
                    h4 = ew.tile([P, EP_N], f16, tag="h4")
                    nc.vector.tensor_tensor(out=h4[:], in0=h3[:], in1=t4[:], op=ALU.add)
                    t5 = ep_silu(ep_mm(6, h4[:]), 7, "t5")
                    t6 = ep_silu(ep_mm(7, t5[:]), 8, "t6")
                    h5 = ew.tile([P, EP_N], f16, tag="h5")
                    nc.vector.tensor_tensor(out=h5[:], in0=h4[:], in1=t6[:], op=ALU.add)
                    # int8 quantization, per-feature scale per 128-col group
                    q8 = ew.tile([P, EP_N], i8, tag="q8")
                    for g in range(EP_N // P):
                        gb = eb * (EP_N // P) + g
                        g0 = g * P
                        amax = ew.tile([P, 1], f32, tag="amax")
                        nc.vector.tensor_reduce(out=amax[:], in_=h5[:, g0:g0 + P],
                                                axis=mybir.AxisListType.X,
                                                op=ALU.max, apply_absolute_value=True)
                        s2a = ew.tile([P, 1], f32, tag="s2a")
                        nc.scalar.activation(out=s2a[:], in_=amax[:], func=AF.Copy,
                                             scale=1.0 / 127.0)
                        s2 = ew.tile([P, 1], f32, tag="s2")
                        nc.vector.tensor_tensor(out=s2[:], in0=s2a[:],
                                                in1=tiny_t[:, 0:1], op=ALU.max)
                        nc.vector.tensor_copy(out=osc_big[:, gb:gb + 1], in_=s2[:])
                        rq = ew.tile([P, 1], f32, tag="rq")
                        nc.vector.reciprocal(out=rq[:], in_=s2[:])
                        nc.scalar.activation(out=q8[:, g0:g0 + P],
                                             in_=h5[:, g0:g0 + P],
                                             func=AF.Copy, scale=rq[:, 0:1])
                    nc.gpsimd.dma_start(out=out_d[:, c0:c0 + EP_N], in_=q8[:])
            nc.gpsimd.dma_start(out=osc_d[:, :], in_=osc_big[:])
    nc.compile()
    return nc


def _build_jitted(nc):
    """Persistent PJRT callable mirroring bass2jax.run_bass_via_pjrt."""
    import jax
    import jax.numpy as jnp
    from jax.sharding import Mesh, PartitionSpec, NamedSharding
    from jax.experimental.shard_map import shard_map
    from concourse import mybir
    from concourse.bass2jax import (_bass_exec_p, partition_id_tensor,
                                    install_neuronx_cc_hook)

    install_neuronx_cc_hook()
    partition_name = nc.partition_id_tensor.name if nc.partition_id_tensor else None
    in_names, out_names, out_avals, out_shapes = [], [], [], []
    for alloc in nc.m.functions[0].allocations:
        if not isinstance(alloc, mybir.MemoryLocationSet):
            continue
        if alloc.kind not in ("ExternalInput", "ExternalOutput"):
            continue
        name = alloc.memorylocations[0].name
        if alloc.kind == "ExternalInput":
            if name != partition_name:
                in_names.append(name)
        else:
            out_names.append(name)
            shape = tuple(alloc.tensor_shape)
            dtype = mybir.dt.np(alloc.dtype)
            out_avals.append(jax.core.ShapedArray(shape, dtype))
            out_shapes.append((shape, dtype))
    n_params = len(in_names)
    all_in = list(in_names) + list(out_names)
    if partition_name is not None:
        all_in.append(partition_name)
    donate = tuple(range(n_params, n_params + len(out_avals)))

    def _body(*args):
        operands = list(args)
        if partition_name is not None:
            operands.append(partition_id_tensor())
        outs = _bass_exec_p.bind(
            *operands, out_avals=tuple(out_avals), in_names=tuple(all_in),
            out_names=tuple(out_names), lowering_input_output_aliases=(),
            sim_require_finite=True, sim_require_nnan=True, nc=nc)
        return tuple(outs)

    devices = jax.devices()[:N_CORES]
    mesh = Mesh(np.asarray(devices), ("core",))
    in_specs = (PartitionSpec("core"),) * (n_params + len(out_avals))
    out_specs = (PartitionSpec("core"),) * len(out_names)
    jitted = jax.jit(shard_map(_body, mesh=mesh, in_specs=in_specs,
                               out_specs=out_specs, check_rep=False),
                     donate_argnums=donate, keep_unused=True)
    sh = NamedSharding(mesh, PartitionSpec("core"))
    mkzeros = jax.jit(
        lambda: tuple(jnp.zeros((N_CORES * s[0], *s[1:]), d) for s, d in out_shapes),
        out_shardings=tuple([sh] * len(out_shapes)))
    return dict(jitted=jitted, mkzeros=mkzeros, in_names=in_names,
                out_names=out_names, out_shapes=out_shapes)


def _exec_cached(cache, in_maps):
    concat_in = [np.concatenate([m[n] for m in in_maps], axis=0)
                 for n in cache["in_names"]]
    zeros = cache["mkzeros"]()
    out_arrs = cache["jitted"](*concat_in, *zeros)
    fetched = [np.asarray(a) for a in out_arrs]
    results = []
    for c in range(N_CORES):
        om = {}
        for i, n in enumerate(cache["out_names"]):
            shape = cache["out_shapes"][i][0]
            om[n] = fetched[i].reshape(N_CORES, *shape)[c]
        results.append(om)
    return results


def kernel(x, radial_basis, spherical_basis, edge_index_from, edge_index_to,
           w_rbf, w_sbf, w_from, b_from, w_to, b_to, W,
           rb_w, rb_b, lin_w, lin_b, ra_w, ra_b):
    from concourse.bass_utils import run_bass_kernel_spmd

    in_dtype = np.asarray(x).dtype
    x = np.asarray(x, np.float32)
    radial = np.asarray(radial_basis, np.float32)
    sph = np.asarray(spherical_basis, np.float32)
    e_from = np.asarray(edge_index_from).astype(np.int64)
    e_to = np.asarray(edge_index_to).astype(np.int64)
    assert x.shape[0] == E_TOT and x.shape[1] == H

    sbf_all = sph @ np.asarray(w_sbf, np.float32)
    cores, meta = host_prep(x, radial, sbf_all, e_from, e_to)
    NB, W_S, NSUB = meta["NB"], meta["W_S"], meta["NSUB"]

    W_np = np.asarray(W, np.float32)
    W2 = np.ascontiguousarray(W_np.transpose(2, 1, 0).reshape(H, B * H)).copy()
    W2 *= np.repeat(meta["sbf_scale"].astype(np.float32), H)[None, :]
    ep_w = np.concatenate([
        np.asarray(w_to, np.float32),
        np.asarray(rb_w, np.float32)[0, 0], np.asarray(rb_w, np.float32)[0, 1],
        np.asarray(lin_w, np.float32),
        np.asarray(ra_w, np.float32)[0, 0], np.asarray(ra_w, np.float32)[0, 1],
        np.asarray(ra_w, np.float32)[1, 0], np.asarray(ra_w, np.float32)[1, 1],
    ], axis=1)
    biases = np.stack([
        np.asarray(b_from, np.float32), np.asarray(b_to, np.float32),
        np.asarray(rb_b, np.float32)[0, 0], np.asarray(rb_b, np.float32)[0, 1],
        np.asarray(lin_b, np.float32),
        np.asarray(ra_b, np.float32)[0, 0], np.asarray(ra_b, np.float32)[0, 1],
        np.asarray(ra_b, np.float32)[1, 0], np.asarray(ra_b, np.float32)[1, 1],
    ], axis=1).astype(np.float32)

    CW16 = P + B * H + 8 * H + P
    c16 = np.zeros((P, CW16), np.float16)
    c16[:, 0:P] = np.asarray(w_from, np.float16)
    c16[:, P:P + B * H] = W2.astype(np.float16)
    c16[:, P + B * H:P + B * H + 8 * H] = ep_w.astype(np.float16)
    c16[0:NR, P + B * H + 8 * H:CW16] = np.asarray(w_rbf, np.float16)
    c32 = np.zeros((P, P + 9), np.float32)
    c32[:, 0:P] = np.tile(np.asarray(b_from, np.float32), (P, 1))
    c32[:, P:P + 9] = biases
    c16 = np.ascontiguousarray(c16)
    c32 = np.ascontiguousarray(c32)

    in_maps = [{
        "c16sh": np.ascontiguousarray(c16[16 * c:16 * c + 16]),
        "c32sh": np.ascontiguousarray(c32[16 * c:16 * c + 16]),
        "sbf": core["sbf"], "tol": core["tol"], "idx": core["idx16"],
        "x_q": core["x_q"], "xsc": core["xsc"], "rad_slots": core["rad_slots"],
    } for c, core in enumerate(cores)]

    key = (NB, W_S, NSUB)
    cache = _CACHE.get(key)
    if cache is None:
        nc = build_program(NB, W_S, NSUB)
        res = run_bass_kernel_spmd(nc, in_maps, core_ids=list(range(N_CORES)))
        kernel._last_results = res
        results = res.results
        cache = _build_jitted(nc)
        cache["nc"] = nc
        _CACHE[key] = cache
        _exec_cached(cache, in_maps)              # warm jit trace/lowering
        if os.environ.get("KERNEL_EXEC_TWICE"):
            import time as _time
            t0 = _time.perf_counter()
            results = _exec_cached(cache, in_maps)
            kernel._exec2_s = _time.perf_counter() - t0
    else:
        results = _exec_cached(cache, in_maps)
        kernel._last_results = None

    out = np.zeros((E_TOT, H), np.float32)
    for core, om in zip(cores, results):
        q = om["out_q"].astype(np.float32)              # [H, W_S]
        sc = om["out_sc"]                                # [H, NB]
        hT = q * np.repeat(sc, P, axis=1)
        out[core["lo"]:core["hi"]] = hT[:, core["slot_local"]].T
    return out.astype(in_dtype, copy=False)


# revision 6
# speedup vs baseline: 1.2106x; 1.2106x over previous
"""Trainium2 Bass kernel (v7: + donated output buffers, async D2H fetch) for DimeNet-style Interaction block (gnn_message_passing).

v3 = v2 (AllGather + device-side dma_gather, fp16 transfers) plus:
  - iota / f32-identity embedded in the NEFF via inline_tensor (no H2D)
  - module-level cache of the compiled program AND the jitted PJRT callable:
    repeat executions skip jit re-tracing (~1.7s) and create the donated
    zero output buffers on-device instead of shipping 27MB of zeros H2D.

Transfer budget per exec: ~42MB H2D + ~27MB D2H over the ~45MB/s axon
tunnel -> ~1.5s steady-state (baseline shipped ~385MB -> 7.5s).
"""
import os
import numpy as np

H, B, NR, NS = 128, 8, 6, 7
P = 128
N_CORES = 8
E_TOT = 100000
EPC = E_TOT // N_CORES          # 12500 edges per core
ROWS_PC = 16384                 # padded x_kj rows per core (AllGather stripe)
CHUNK = 32768                   # dma_gather int16 index range per chunk
EP_N = 512

_CACHE = {}


def host_prep(x, radial, sbf_all, e_from, e_to):
    """Per-core packed tensors + meta. All numpy, vectorized."""
    sbf_scale = np.maximum(np.abs(sbf_all).max(0) / 127.0, 1e-20)  # [B] f64
    perm = np.argsort(e_to, kind="stable")
    tos = e_to[perm].astype(np.int64)
    frs = e_from[perm].astype(np.int64)

    cores = []
    NB_max = 0
    for c in range(N_CORES):
        lo, hi = c * EPC, (c + 1) * EPC
        t0, t1 = np.searchsorted(tos, lo), np.searchsorted(tos, hi)
        lt = tos[t0:t1] - lo
        fr = frs[t0:t1]
        # gather row = core(from)*16384 + slotpos, slotpos < 16384
        # -> chunk(row) = row >> 15 = core(from) // 2 exactly.
        chk = (fr // EPC) // 2
        cnt = np.bincount(lt * 4 + chk, minlength=EPC * 4).reshape(EPC, 4)
        Ccum = np.concatenate([np.zeros((1, 4), np.int64), np.cumsum(cnt, 0)], 0)
        s, starts = 0, []
        while s < EPC:
            ends = [np.searchsorted(Ccum[:, k], Ccum[s, k] + P, side="right") - 1
                    for k in range(4)]
            end = min(s + P, *ends)
            assert end > s, "single edge exceeds 128 triplets in one chunk"
            starts.append(s)
            s = end
        starts = np.asarray(starts + [EPC], np.int64)
        NB_max = max(NB_max, len(starts) - 1)
        cores.append(dict(lo=lo, hi=hi, lt=lt, fr=fr, chk=chk,
                          tsl=perm[t0:t1], starts=starts, nb=len(starts) - 1))

    NB = -(-NB_max // 8) * 8
    W_S = NB * P
    NSUB = 4 * NB

    slotpos = np.empty(E_TOT, np.int64)
    for core in cores:
        starts, nb = core["starts"], core["nb"]
        widths = np.diff(starts)
        blk_of_edge = np.repeat(np.arange(nb), widths)
        cov_lo = starts[:-1]
        sp = blk_of_edge * P + (np.arange(EPC) - cov_lo[blk_of_edge])
        slotpos[core["lo"]:core["hi"]] = sp
        core["blk_of_edge"] = blk_of_edge
        core["cov_lo"] = cov_lo
        core["slot_local"] = sp
    grow = (np.arange(E_TOT) // EPC) * ROWS_PC + slotpos

    for core in cores:
        lt, fr, tsl = core["lt"], core["fr"], core["tsl"]
        blk = core["blk_of_edge"][lt]
        gid = blk * 4 + core["chk"]
        order = np.argsort(gid, kind="stable")
        gid_s = gid[order]
        first_idx = np.concatenate([[0], np.flatnonzero(np.diff(gid_s)) + 1])
        counts = np.diff(np.concatenate([first_idx, [len(gid_s)]]))
        assert counts.max() <= P, counts.max()
        rank = np.arange(len(gid_s)) - np.repeat(first_idx, counts)
        dst = gid_s * P + rank
        tri = order
        idx_arr = np.zeros(NSUB * P, np.int16)
        tol_arr = np.full(NSUB * P, 255.0, np.float16)
        sbf_arr = np.zeros((NSUB * P, B), np.int8)
        idx_arr[dst] = (grow[fr[tri]] & (CHUNK - 1)).astype(np.int16)
        tol_arr[dst] = (lt[tri] - core["cov_lo"][blk[tri]]).astype(np.float16)
        sbf_arr[dst] = np.clip(np.round(sbf_all[tsl[tri]] / sbf_scale[None, :]),
                               -127, 127).astype(np.int8)
        core["idx16"] = np.ascontiguousarray(
            idx_arr.reshape(NSUB, 8, 16).transpose(2, 0, 1).reshape(16, NSUB * 8))
        core["tol"] = np.ascontiguousarray(tol_arr.reshape(NSUB, P).T)
        core["sbf"] = np.ascontiguousarray(
            sbf_arr.reshape(NSUB, P, B).transpose(1, 0, 2).reshape(P, NSUB * B))
        rs = np.zeros((W_S, NR), np.float16)
        sl = core["slot_local"]
        rs[sl] = radial[core["lo"]:core["hi"]].astype(np.float16)
        core["rad_slots"] = np.ascontiguousarray(rs.T)
        # int8 x with per-feature-per-block scales (dequantized on device)
        xc = x[core["lo"]:core["hi"]]                    # [EPC, H]
        xf = np.zeros((W_S, H), np.float32)
        xf[sl] = xc
        xf3 = xf.reshape(NB, P, H)
        sc = np.maximum(np.abs(xf3).max(1) / 127.0, 1e-20).astype(np.float32)  # [NB, H]
        xq3 = np.clip(np.round(xf3 / sc[:, None, :]), -127, 127).astype(np.int8)
        core["x_q"] = np.ascontiguousarray(xq3.reshape(W_S, H).T)   # [H, W_S]
        core["xsc"] = np.ascontiguousarray(sc.T)         # [H, NB] f32
    return cores, dict(NB=NB, W_S=W_S, NSUB=NSUB, sbf_scale=sbf_scale)


def build_program(NB, W_S, NSUB):
    import concourse.tile as tile
    from concourse import bacc, mybir

    f32 = mybir.dt.float32
    f16 = mybir.dt.float16
    i16 = mybir.dt.int16
    AF = mybir.ActivationFunctionType
    ALU = mybir.AluOpType

    nc = bacc.Bacc(None, target_bir_lowering=False)
    CW16 = P + B * H + 8 * H + P           # w_from|W2|epw|w_rbf
    i8 = mybir.dt.int8
    c16_d = nc.dram_tensor("c16sh", [16, CW16], f16, kind="ExternalInput")
    c32_d = nc.dram_tensor("c32sh", [16, P + 9], f32, kind="ExternalInput")  # b_bcast|biases
    sbf_d = nc.dram_tensor("sbf", [P, B * NSUB], mybir.dt.int8, kind="ExternalInput")
    tol_d = nc.dram_tensor("tol", [P, NSUB], f16, kind="ExternalInput")
    idx_d = nc.dram_tensor("idx", [16, 8 * NSUB], i16, kind="ExternalInput")
    xq_d = nc.dram_tensor("x_q", [P, W_S], i8, kind="ExternalInput")
    xsc_d = nc.dram_tensor("xsc", [P, NB], f32, kind="ExternalInput")
    rad_d = nc.dram_tensor("rad_slots", [NR, W_S], f16, kind="ExternalInput")
    out_d = nc.dram_tensor("out_q", [P, W_S], i8, kind="ExternalOutput")
    osc_d = nc.dram_tensor("out_sc", [P, NB], f32, kind="ExternalOutput")
    iota_c = nc.inline_tensor(
        np.tile(np.arange(P, dtype=np.float16), (P, 1)), name="iota_c")
    tiny_c = nc.inline_tensor(np.full((P, 1), 1e-12, np.float32), name="tiny_c")
    idf32_c = nc.inline_tensor(np.eye(P, dtype=np.float32), name="idf32_c")

    with tile.TileContext(nc) as tc:
        with (
            tc.tile_pool(name="consts", bufs=1) as cp,
            tc.tile_pool(name="persist", bufs=1) as pp,
            tc.tile_pool(name="dram", bufs=1, space="DRAM") as dr,
        ):
            # weights arrive sharded (1/8 rows per core): AllGather on device
            c16b = dr.tile([16, CW16], f16)
            nc.gpsimd.dma_start(out=c16b[:, :], in_=c16_d[:, :])
            c16f = dr.tile([P, CW16], f16)
            nc.gpsimd.collective_compute(
                "AllGather", mybir.AluOpType.bypass,
                replica_groups=[list(range(N_CORES))],
                ins=[c16b[:, :].opt()], outs=[c16f[:, :].opt()])
            c32b = dr.tile([16, P + 9], f32)
            nc.gpsimd.dma_start(out=c32b[:, :], in_=c32_d[:, :])
            c32f = dr.tile([P, P + 9], f32)
            nc.gpsimd.collective_compute(
                "AllGather", mybir.AluOpType.bypass,
                replica_groups=[list(range(N_CORES))],
                ins=[c32b[:, :].opt()], outs=[c32f[:, :].opt()])
            c16 = cp.tile([P, CW16], f16)
            nc.gpsimd.dma_start(out=c16[:], in_=c16f[:, :])
            c32 = cp.tile([P, P + 9], f32)
            nc.gpsimd.dma_start(out=c32[:], in_=c32f[:, :])
            iota_t = cp.tile([P, P], f16)
            nc.gpsimd.dma_start(out=iota_t[:], in_=iota_c[:, :])
            idf32_t = cp.tile([P, P], f32)
            nc.gpsimd.dma_start(out=idf32_t[:], in_=idf32_c[:, :])
            sbf_t = cp.tile([P, B * NSUB], mybir.dt.int8)
            nc.gpsimd.dma_start(out=sbf_t[:], in_=sbf_d[:, :])
            tol_t = cp.tile([P, NSUB], f16)
            nc.gpsimd.dma_start(out=tol_t[:], in_=tol_d[:, :])
            xq_t = cp.tile([P, W_S], i8)
            nc.gpsimd.dma_start(out=xq_t[:], in_=xq_d[:, :])
            xsc_t = cp.tile([P, NB], f32)
            nc.gpsimd.dma_start(out=xsc_t[:], in_=xsc_d[:, :])
            xs_t = cp.tile([P, W_S], f16)
            for q in range(NB):
                q0 = q * P
                nc.scalar.activation(out=xs_t[:, q0:q0 + P], in_=xq_t[:, q0:q0 + P],
                                     func=AF.Copy, scale=xsc_t[:, q:q + 1])
            rad_t = cp.tile([NR, W_S], f16)
            nc.gpsimd.dma_start(out=rad_t[:], in_=rad_d[:, :])
            tiny_t = cp.tile([P, 1], f32)
            nc.gpsimd.dma_start(out=tiny_t[:], in_=tiny_c[:, :])
            osc_big = pp.tile([P, NB], f32)
            idx_t = cp.tile([P, 8 * NSUB], i16)
            for k in range(8):
                nc.gpsimd.dma_start(out=idx_t[16 * k:16 * k + 16, :], in_=idx_d[:, :])

            w_from_t = c16[:, 0:P]
            W2_t = c16[:, P:P + B * H]
            epw_t = c16[:, P + B * H:P + B * H + 8 * H]
            w_rbf_t = c16[0:NR, P + B * H + 8 * H:CW16]
            b_bcast = c32[:, 0:P]
            bias_t = c32[:, P:P + 9]

            aggT_big = pp.tile([P, W_S], f16)
            xkj_mine = dr.tile([ROWS_PC, H], f16)
            xkj_all = dr.tile([ROWS_PC * N_CORES, H], f16)

            # ---------------- stage 1: x_kj rows ----------------
            with (
                tc.tile_pool(name="s1w", bufs=3) as s1w,
                tc.tile_pool(name="s1p", bufs=2, space="PSUM") as s1p,
            ):
                for b in range(NB):
                    c0 = b * P
                    xw_p = s1p.tile([P, P], f32, tag="xw")
                    nc.tensor.matmul(out=xw_p[:], lhsT=xs_t[:, c0:c0 + P],
                                     rhs=w_from_t, start=True, stop=True)
                    rb_p = s1p.tile([P, P], f32, tag="rb")
                    nc.tensor.matmul(out=rb_p[:], lhsT=rad_t[:, c0:c0 + P],
                                     rhs=w_rbf_t, start=True, stop=True)
                    t1 = s1w.tile([P, P], f32, tag="t1")
                    nc.vector.tensor_tensor(out=t1[:], in0=xw_p[:], in1=b_bcast,
                                            op=ALU.add)
                    t2 = s1w.tile([P, P], f32, tag="t2")
                    nc.scalar.activation(out=t2[:], in_=t1[:], func=AF.Silu)
                    xkj = s1w.tile([P, P], f16, tag="xkj")
                    nc.vector.tensor_tensor(out=xkj[:], in0=t2[:], in1=rb_p[:],
                                            op=ALU.mult)
                    nc.gpsimd.dma_start(out=xkj_mine[c0:c0 + P, :], in_=xkj[:])

            # ---------------- AllGather ----------------
            nc.gpsimd.collective_compute(
                "AllGather", mybir.AluOpType.bypass,
                replica_groups=[list(range(N_CORES))],
                ins=[xkj_mine[:, :].opt()], outs=[xkj_all[:, :].opt()],
            )

            # ---------------- main loop ----------------
            with (
                tc.tile_pool(name="mg", bufs=4) as mg,
                tc.tile_pool(name="mw", bufs=3) as mw,
                tc.tile_pool(name="ptmp", bufs=1, space="PSUM") as ptmp,
                tc.tile_pool(name="pagg", bufs=2, space="PSUM") as pagg,
                tc.tile_pool(name="ptr", bufs=1, space="PSUM") as ptr,
            ):
                for b in range(NB):
                    agg_p = pagg.tile([P, P], f32, tag="agg")
                    for s in range(4):
                        sub = b * 4 + s
                        g3 = mg.tile([P, 1, P], f16, tag="g")
                        nc.gpsimd.dma_gather(
                            out_ap=g3[:],
                            in_ap=xkj_all[s * CHUNK:(s + 1) * CHUNK, :],
                            idxs_ap=idx_t[:, sub * 8:sub * 8 + 8],
                            num_idxs=P, num_idxs_reg=P,
                            elem_size=H, transpose=True)
                        g = g3[:].squeeze(1)
                        tmpA = ptmp.tile([P, 4 * H], f32, tag="tmpA")
                        nc.tensor.matmul(out=tmpA[:], lhsT=g,
                                         rhs=W2_t[:, 0:4 * H], start=True, stop=True)
                        tmpB = ptmp.tile([P, 4 * H], f32, tag="tmpB")
                        nc.tensor.matmul(out=tmpB[:], lhsT=g,
                                         rhs=W2_t[:, 4 * H:8 * H], start=True, stop=True)
                        sc = mw.tile([P, B], f32, tag="sc")
                        nc.scalar.activation(out=sc[:],
                                             in_=sbf_t[:, sub * B:(sub + 1) * B],
                                             func=AF.Copy)
                        S = mw.tile([P, P], f16, tag="S")
                        nc.vector.tensor_tensor(
                            out=S[:],
                            in0=tol_t[:, sub:sub + 1].to_broadcast([P, P]),
                            in1=iota_t[:], op=ALU.is_equal)
                        tmpS = mw.tile([P, B * H], f16, tag="tmpS")
                        for j in range(B):
                            src = tmpA[:, j * H:(j + 1) * H] if j < 4 else \
                                  tmpB[:, (j - 4) * H:(j - 3) * H]
                            dstp = tmpS[:, j * H:(j + 1) * H]
                            scj = sc[:, j:j + 1]
                            if j % 2 == 0:
                                nc.scalar.activation(out=dstp, in_=src, func=AF.Copy,
                                                     scale=scj)
                            else:
                                nc.vector.tensor_tensor(
                                    out=dstp, in0=src,
                                    in1=scj.to_broadcast([P, H]), op=ALU.mult)
                        for j in range(B):
                            nc.tensor.matmul(out=agg_p[:], lhsT=S[:],
                                             rhs=tmpS[:, j * H:(j + 1) * H],
                                             start=(s == 0 and j == 0),
                                             stop=(s == 3 and j == B - 1),
                                             skip_group_check=True)
                    agg_s = mw.tile([P, P], f32, tag="agg_s")
                    nc.scalar.activation(out=agg_s[:], in_=agg_p[:], func=AF.Copy)
                    aggT_p = ptr.tile([P, P], f32, tag="aggT")
                    nc.tensor.transpose(out=aggT_p[:], in_=agg_s[:], identity=idf32_t[:])
                    nc.vector.tensor_copy(out=aggT_big[:, b * P:(b + 1) * P],
                                          in_=aggT_p[:])

            # ---------------- epilogue ----------------
            with (
                tc.tile_pool(name="ew", bufs=2) as ew,
                tc.tile_pool(name="ep", bufs=4, space="PSUM") as ep,
            ):
                def ep_mm(lhs_idx, rhs_ap):
                    pt = ep.tile([P, EP_N], f32, tag="ep_p")
                    nc.tensor.matmul(out=pt[:],
                                     lhsT=epw_t[:, lhs_idx * H:(lhs_idx + 1) * H],
                                     rhs=rhs_ap, start=True, stop=True)
                    return pt

                def ep_silu(pt, bias_idx, tag):
                    t = ew.tile([P, EP_N], f16, tag=tag)
                    nc.scalar.activation(out=t[:], in_=pt[:], func=AF.Silu,
                                         bias=bias_t[:, bias_idx:bias_idx + 1],
                                         scale=1.0)
                    return t

                for eb in range(W_S // EP_N):
                    c0 = eb * EP_N
                    x_sl = xs_t[:, c0:c0 + EP_N]
                    xji = ep_silu(ep_mm(0, x_sl), 1, "xji")
                    h = ew.tile([P, EP_N], f16, tag="h")
                    nc.vector.tensor_tensor(out=h[:], in0=xji[:],
                                            in1=aggT_big[:, c0:c0 + EP_N], op=ALU.add)
                    t1 = ep_silu(ep_mm(1, h[:]), 2, "t1")
                    t2 = ep_silu(ep_mm(2, t1[:]), 3, "t2")
                    h2 = ew.tile([P, EP_N], f16, tag="h2")
                    nc.vector.tensor_tensor(out=h2[:], in0=h[:], in1=t2[:], op=ALU.add)
                    l1 = ep_silu(ep_mm(3, h2[:]), 4, "l1")
                    h3 = ew.tile([P, EP_N], f16, tag="h3")
                    nc.vector.tensor_tensor(out=h3[:], in0=l1[:], in1=x_sl, op=ALU.add)
                    t3 = ep_silu(ep_mm(4, h3[:]), 5, "t3")
                    t4 = ep_silu(ep_mm(5, t3[:]), 6, "t4")
                    h4 = ew.tile([P, EP_N], f16, tag="h4")
                    nc.vector.tensor_tensor(out=h4[:], in0=h3[:], in1=t4[:], op=ALU.add)
                    t5 = ep_silu(ep_mm(6, h4[:]), 7, "t5")
                    t6 = ep_silu(ep_mm(7, t5[:]), 8, "t6")
                    h5 = ew.tile([P, EP_N], f16, tag="h5")
                    nc.vector.tensor_tensor(out=h5[:], in0=h4[:], in1=t6[:], op=ALU.add)
                    # int8 quantization, per-feature scale per 128-col group
                    q8 = ew.tile([P, EP_N], i8, tag="q8")
                    for g in range(EP_N // P):
                        gb = eb * (EP_N // P) + g
                        g0 = g * P
                        amax = ew.tile([P, 1], f32, tag="amax")
                        nc.vector.tensor_reduce(out=amax[:], in_=h5[:, g0:g0 + P],
                                                axis=mybir.AxisListType.X,
                                                op=ALU.max, apply_absolute_value=True)
                        s2a = ew.tile([P, 1], f32, tag="s2a")
                        nc.scalar.activation(out=s2a[:], in_=amax[:], func=AF.Copy,
                                             scale=1.0 / 127.0)
                        s2 = ew.tile([P, 1], f32, tag="s2")
                        nc.vector.tensor_tensor(out=s2[:], in0=s2a[:],
                                                in1=tiny_t[:, 0:1], op=ALU.max)
                        nc.vector.tensor_copy(out=osc_big[:, gb:gb + 1], in_=s2[:])
                        rq = ew.tile([P, 1], f32, tag="rq")
                        nc.vector.reciprocal(out=rq[:], in_=s2[:])
                        nc.scalar.activation(out=q8[:, g0:g0 + P],
                                             in_=h5[:, g0:g0 + P],
                                             func=AF.Copy, scale=rq[:, 0:1])
                    nc.gpsimd.dma_start(out=out_d[:, c0:c0 + EP_N], in_=q8[:])
            nc.gpsimd.dma_start(out=osc_d[:, :], in_=osc_big[:])
    nc.compile()
    return nc


def _build_jitted(nc):
    """Persistent PJRT callable mirroring bass2jax.run_bass_via_pjrt."""
    import jax
    import jax.numpy as jnp
    from jax.sharding import Mesh, PartitionSpec, NamedSharding
    from jax.experimental.shard_map import shard_map
    from concourse import mybir
    from concourse.bass2jax import (_bass_exec_p, partition_id_tensor,
                                    install_neuronx_cc_hook)

    install_neuronx_cc_hook()
    partition_name = nc.partition_id_tensor.name if nc.partition_id_tensor else None
    in_names, out_names, out_avals, out_shapes = [], [], [], []
    for alloc in nc.m.functions[0].allocations:
        if not isinstance(alloc, mybir.MemoryLocationSet):
            continue
        if alloc.kind not in ("ExternalInput", "ExternalOutput"):
            continue
        name = alloc.memorylocations[0].name
        if alloc.kind == "ExternalInput":
            if name != partition_name:
                in_names.append(name)
        else:
            out_names.append(name)
            shape = tuple(alloc.tensor_shape)
            dtype = mybir.dt.np(alloc.dtype)
            out_avals.append(jax.core.ShapedArray(shape, dtype))
            out_shapes.append((shape, dtype))
    n_params = len(in_names)
    all_in = list(in_names) + list(out_names)
    if partition_name is not None:
        all_in.append(partition_name)
    donate = tuple(range(n_params, n_params + len(out_avals)))

    def _body(*args):
        operands = list(args)
        if partition_name is not None:
            operands.append(partition_id_tensor())
        outs = _bass_exec_p.bind(
            *operands, out_avals=tuple(out_avals), in_names=tuple(all_in),
            out_names=tuple(out_names), lowering_input_output_aliases=(),
            sim_require_finite=True, sim_require_nnan=True, nc=nc)
        return tuple(outs)

    devices = jax.devices()[:N_CORES]
    mesh = Mesh(np.asarray(devices), ("core",))
    in_specs = (PartitionSpec("core"),) * (n_params + len(out_avals))
    out_specs = (PartitionSpec("core"),) * len(out_names)
    jitted = jax.jit(shard_map(_body, mesh=mesh, in_specs=in_specs,
                               out_specs=out_specs, check_rep=False),
                     donate_argnums=donate, keep_unused=True)
    sh = NamedSharding(mesh, PartitionSpec("core"))
    mkzeros = jax.jit(
        lambda: tuple(jnp.zeros((N_CORES * s[0], *s[1:]), d) for s, d in out_shapes),
        out_shardings=tuple([sh] * len(out_shapes)))
    return dict(jitted=jitted, mkzeros=mkzeros, in_names=in_names,
                out_names=out_names, out_shapes=out_shapes)


def _exec_cached(cache, in_maps):
    concat_in = [np.concatenate([m[n] for m in in_maps], axis=0)
                 for n in cache["in_names"]]
    # donate the previous exec's output buffers (every output element is
    # rewritten by the program, so initial content is irrelevant)
    prev = cache.pop("prev_outs", None)
    if prev is None:
        prev = cache["mkzeros"]()
    out_arrs = cache["jitted"](*concat_in, *prev)
    cache["prev_outs"] = out_arrs
    for a in out_arrs:
        try:
            for s in a.addressable_shards:
                s.data.copy_to_host_async()
        except Exception:
            pass
    fetched = [np.asarray(a) for a in out_arrs]
    results = []
    for c in range(N_CORES):
        om = {}
        for i, n in enumerate(cache["out_names"]):
            shape = cache["out_shapes"][i][0]
            om[n] = fetched[i].reshape(N_CORES, *shape)[c]
        results.append(om)
    return results


def kernel(x, radial_basis, spherical_basis, edge_index_from, edge_index_to,
           w_rbf, w_sbf, w_from, b_from, w_to, b_to, W,
           rb_w, rb_b, lin_w, lin_b, ra_w, ra_b):
    from concourse.bass_utils import run_bass_kernel_spmd

    in_dtype = np.asarray(x).dtype
    x = np.asarray(x, np.float32)
    radial = np.asarray(radial_basis, np.float32)
    sph = np.asarray(spherical_basis, np.float32)
    e_from = np.asarray(edge_index_from).astype(np.int64)
    e_to = np.asarray(edge_index_to).astype(np.int64)
    assert x.shape[0] == E_TOT and x.shape[1] == H

    sbf_all = sph @ np.asarray(w_sbf, np.float32)
    cores, meta = host_prep(x, radial, sbf_all, e_from, e_to)
    NB, W_S, NSUB = meta["NB"], meta["W_S"], meta["NSUB"]

    W_np = np.asarray(W, np.float32)
    W2 = np.ascontiguousarray(W_np.transpose(2, 1, 0).reshape(H, B * H)).copy()
    W2 *= np.repeat(meta["sbf_scale"].astype(np.float32), H)[None, :]
    ep_w = np.concatenate([
        np.asarray(w_to, np.float32),
        np.asarray(rb_w, np.float32)[0, 0], np.asarray(rb_w, np.float32)[0, 1],
        np.asarray(lin_w, np.float32),
        np.asarray(ra_w, np.float32)[0, 0], np.asarray(ra_w, np.float32)[0, 1],
        np.asarray(ra_w, np.float32)[1, 0], np.asarray(ra_w, np.float32)[1, 1],
    ], axis=1)
    biases = np.stack([
        np.asarray(b_from, np.float32), np.asarray(b_to, np.float32),
        np.asarray(rb_b, np.float32)[0, 0], np.asarray(rb_b, np.float32)[0, 1],
        np.asarray(lin_b, np.float32),
        np.asarray(ra_b, np.float32)[0, 0], np.asarray(ra_b, np.float32)[0, 1],
        np.asarray(ra_b, np.float32)[1, 0], np.asarray(ra_b, np.float32)[1, 1],
    ], axis=1).astype(np.float32)

    CW16 = P + B * H + 8 * H + P
    c16 = np.zeros((P, CW16), np.float16)
    c16[:, 0:P] = np.asarray(w_from, np.float16)
    c16[:, P:P + B * H] = W2.astype(np.float16)
    c16[:, P + B * H:P + B * H + 8 * H] = ep_w.astype(np.float16)
    c16[0:NR, P + B * H + 8 * H:CW16] = np.asarray(w_rbf, np.float16)
    c32 = np.zeros((P, P + 9), np.float32)
    c32[:, 0:P] = np.tile(np.asarray(b_from, np.float32), (P, 1))
    c32[:, P:P + 9] = biases
    c16 = np.ascontiguousarray(c16)
    c32 = np.ascontiguousarray(c32)

    in_maps = [{
        "c16sh": np.ascontiguousarray(c16[16 * c:16 * c + 16]),
        "c32sh": np.ascontiguousarray(c32[16 * c:16 * c + 16]),
        "sbf": core["sbf"], "tol": core["tol"], "idx": core["idx16"],
        "x_q": core["x_q"], "xsc": core["xsc"], "rad_slots": core["rad_slots"],
    } for c, core in enumerate(cores)]

    key = (NB, W_S, NSUB)
    cache = _CACHE.get(key)
    if cache is None:
        nc = build_program(NB, W_S, NSUB)
        res = run_bass_kernel_spmd(nc, in_maps, core_ids=list(range(N_CORES)))
        kernel._last_results = res
        results = res.results
        cache = _build_jitted(nc)
        cache["nc"] = nc
        _CACHE[key] = cache
        _exec_cached(cache, in_maps)              # warm jit trace/lowering
        if os.environ.get("KERNEL_EXEC_TWICE"):
            import time as _time
            t0 = _time.perf_counter()
            results = _exec_cached(cache, in_maps)
            kernel._exec2_s = _time.perf_counter() - t0
    else:
        results = _exec_cached(cache, in_maps)
        kernel._last_results = None

    out = np.zeros((E_TOT, H), np.float32)
    for core, om in zip(cores, results):
        q = om["out_q"].astype(np.float32)              # [H, W_S]
        sc = om["out_sc"]                                # [H, NB]
        hT = q * np.repeat(sc, P, axis=1)
        out[core["lo"]:core["hi"]] = hT[:, core["slot_local"]].T
    return out.astype(in_dtype, copy=False)
